# revision 1
# baseline (speedup 1.0000x reference)
# Trainium2 Bass kernel for DCNNv2 GNN message passing.
#
# Strategy (per spec sharding hint): shard the G (graph) axis data-parallel
# across 8 cores; replicate the 10000x128 impact table and the small weights.
# On each core:
#   Phase A: T = [impact @ M.T ; impact @ W.T]  (20000x128 f32, in local DRAM)
#   Phase B: one dma_gather stream per chunk of 1024 (g,k) nodes: 8 neighbor
#            rows (from T_M) + 1 self row (from T_W) per node, j-major layout.
#            PE identity-matmul accumulates the 9 rows per node into PSUM,
#            ACT applies relu, PE ones-matmul sums the 64 nodes per graph,
#            softmax -> E shard.
#   AllGather E -> E_full (Shared DRAM)
#   Phase D: gather E_full[ext_nbr], PE-reduce, U/V matmuls, relu, softmax -> X.
#   AllGather X -> X_full
#   Phase E: link prediction on a 128-pair shard of the batch.
# Host side only marshals data (sharding, index int16 packing, transposes of
# weight matrices); all FLOPs happen on device.

import numpy as np

D = 128
NT = 10000       # impact rows
G = 2000
K = 64
DIN = 8
DEXT = 16
B = 1024
NCORES = 8
GL = G // NCORES           # 250 graphs per core
NKL = GL * K               # 16000 (g,k) rows per core
CHUNK = 1024               # gk rows per gather chunk
NSTREAM = DIN + 1          # 8 neighbor slots + self
BL = B // NCORES           # 128 batch pairs per core

_PROGRAM_CACHE = {}


def _chunks():
    out = []
    lo = 0
    while lo < NKL:
        hi = min(lo + CHUNK, NKL)
        out.append((lo, hi))
        lo = hi
    return out


def _idx_cols(n):
    return n * NSTREAM // 16   # int16 idx columns for n gk rows


def _wrap16(flat_i16):
    """Pack a flat int16 index stream for dma_gather: element i at
    [i % 16, i // 16], replicated across the 8 groups of 16 partitions."""
    a = np.asarray(flat_i16, dtype=np.int16).reshape(-1, 16).T   # [16, n/16]
    return np.ascontiguousarray(np.tile(a, (8, 1)))              # [128, n/16]


def build_program():
    import concourse.bacc as bacc
    import concourse.tile as tile
    import concourse.mybir as mybir

    f32 = mybir.dt.float32
    i16 = mybir.dt.int16
    AF = mybir.ActivationFunctionType
    ALU = mybir.AluOpType

    nc = bacc.Bacc(
        "TRN2",
        target_bir_lowering=False,
        debug=False,
        enable_asserts=False,
        num_devices=NCORES,
    )

    # ---- external inputs (per core) ----
    impact_T = nc.dram_tensor("impact_T", [D, NT], f32, kind="ExternalInput").ap()
    rhs_MW = nc.dram_tensor("rhs_MW", [D, 2 * D], f32, kind="ExternalInput").ap()
    UT = nc.dram_tensor("UT", [D, D], f32, kind="ExternalInput").ap()
    VT = nc.dram_tensor("VT", [D, D], f32, kind="ExternalInput").ap()
    W1mT = nc.dram_tensor("W1mT", [D, D], f32, kind="ExternalInput").ap()
    W1sT = nc.dram_tensor("W1sT", [D, D], f32, kind="ExternalInput").ap()
    W2T = nc.dram_tensor("W2T", [D, 2], f32, kind="ExternalInput").ap()
    b1_in = nc.dram_tensor("b1", [D, 1], f32, kind="ExternalInput").ap()
    b2_in = nc.dram_tensor("b2", [2, 1], f32, kind="ExternalInput").ap()
    ident_in = nc.dram_tensor("ident", [D, D], f32, kind="ExternalInput").ap()
    ks_in = nc.dram_tensor("ks", [D, 8 * 16], f32, kind="ExternalInput").ap()

    n_big_cols = sum(_idx_cols(hi - lo) for lo, hi in _chunks())
    idx_big_in = nc.dram_tensor("idx_big", [D, n_big_cols], i16, kind="ExternalInput").ap()
    idx_ext_in = nc.dram_tensor("idx_ext", [D, 256], i16, kind="ExternalInput").ap()
    idx_pair_in = nc.dram_tensor("idx_pair", [D, 16], i16, kind="ExternalInput").ap()

    out_dram = nc.dram_tensor("out", [BL, 2], f32, kind="ExternalOutput").ap()

    with tile.TileContext(nc) as tc:
        # ---- long-lived DRAM scratch ----
        T_M_dram, _f0m = tc.tile([NT, D], f32, space="DRAM", name="T_M_table")
        T_W_dram, _f0w = tc.tile([NT, D], f32, space="DRAM", name="T_W_table")
        E_loc_dram, _f1 = tc.tile([GL, D], f32, space="DRAM", name="E_loc")
        E_full, _f2 = tc.tile([G, D], f32, space="DRAM", addr_space="Shared",
                              name="E_full")
        X_loc_dram, _f3 = tc.tile([GL, D], f32, space="DRAM", name="X_loc")
        X_full, _f4 = tc.tile([G, D], f32, space="DRAM", addr_space="Shared",
                              name="X_full")

        # ---- long-lived SBUF constants ----
        cpool_cm = tc.tile_pool(name="consts", bufs=1)
        cpool = cpool_cm.__enter__()
        ident_sb = cpool.tile([D, D], f32, name="ident_sb")
        nc.sync.dma_start(out=ident_sb[:], in_=ident_in[:])
        ks_sb = cpool.tile([D, 8 * 16], f32, name="ks_sb")
        nc.sync.dma_start(out=ks_sb[:], in_=ks_in[:])
        idx_big_sb = cpool.tile([D, n_big_cols], i16, name="idx_big_sb")
        nc.sync.dma_start(out=idx_big_sb[:], in_=idx_big_in[:])

        # =========================== Phase A ===========================
        with tc.tile_pool(name="phaseA_sb", bufs=3) as apool, \
             tc.tile_pool(name="phaseA_ps", bufs=4, space="PSUM") as appool, \
             tc.tile_pool(name="phaseA_imp", bufs=3) as ipool:
            mw_sb = apool.tile([D, 2 * D], f32, name="mw_sb")
            nc.sync.dma_start(out=mw_sb[:], in_=rhs_MW[:])

            n_tiles = (NT + D - 1) // D        # 79
            GRP = 8
            t = 0
            while t < n_tiles:
                ns = min(GRP, n_tiles - t)
                gw = min(ns * D, NT - t * D)
                imp_g = ipool.tile([D, GRP * D], f32, tag="impg")
                nc.sync.dma_start(out=imp_g[:, :gw],
                                  in_=impact_T[:, t * D:t * D + gw])
                stage = apool.tile([D, ns, 2 * D], f32, tag="stageA")
                for s in range(ns):
                    tw = min(D, NT - (t + s) * D)      # 128, last tile 16
                    psA = appool.tile([D, 2 * D], f32, tag="psA")
                    nc.tensor.matmul(
                        out=psA[:tw, :],
                        lhsT=imp_g[:, s * D:s * D + tw],
                        rhs=mw_sb[:],
                        start=True, stop=True,
                    )
                    if s % 2 == 0:
                        nc.scalar.copy(out=stage[:tw, s, :], in_=psA[:tw, :])
                    else:
                        nc.vector.tensor_copy(out=stage[:tw, s, :], in_=psA[:tw, :])
                # full 128-row tiles in this group
                nf = ns if (t + ns) * D <= NT else ns - 1
                base = t * D
                if nf > 0:
                    nrows = nf * D
                    nc.sync.dma_start(
                        out=T_M_dram[base:base + nrows, :]
                            .rearrange("(s p) d -> p s d", p=D),
                        in_=stage[:, :nf, 0:D],
                    )
                    nc.sync.dma_start(
                        out=T_W_dram[base:base + nrows, :]
                            .rearrange("(s p) d -> p s d", p=D),
                        in_=stage[:, :nf, D:2 * D],
                    )
                if nf < ns:  # partial last tile (16 rows)
                    pb = base + nf * D
                    pw = NT - pb
                    nc.sync.dma_start(out=T_M_dram[pb:pb + pw, :],
                                      in_=stage[:pw, nf, 0:D])
                    nc.sync.dma_start(out=T_W_dram[pb:pb + pw, :],
                                      in_=stage[:pw, nf, D:2 * D])
                t += ns

        # =========================== Phase B ===========================
        with tc.tile_pool(name="gpool", bufs=2) as gpool, \
             tc.tile_pool(name="bpool", bufs=3) as bpool, \
             tc.tile_pool(name="bpsum", bufs=3, space="PSUM") as bppool, \
             tc.tile_pool(name="b2psum", bufs=2, space="PSUM") as b2ppool:
            col0 = 0
            for ci, (lo, hi) in enumerate(_chunks()):
                nb = hi - lo
                nblk = nb // D                     # 8 (last chunk 5)
                ncols = _idx_cols(nb)
                gt = gpool.tile([D, NSTREAM * nblk, D], f32, tag="gt")
                jcols = nb // 16          # idx cols per j-stream (<= 64)
                for j in range(NSTREAM):
                    nc.gpsimd.dma_gather(
                        out_ap=gt[:, j * nblk:(j + 1) * nblk, :],
                        in_ap=(T_M_dram[:] if j < DIN else T_W_dram[:]),
                        idxs_ap=idx_big_sb[:, col0 + j * jcols:
                                           col0 + (j + 1) * jcols],
                        num_idxs=nb,
                        num_idxs_reg=nb,
                        elem_size=D,
                    )
                col0 += ncols

                ps2 = b2ppool.tile([16, D], f32, tag="ps2")
                for h in range(0, nblk, 4):
                    hw = min(4, nblk - h)
                    ps = bppool.tile([D, 4 * D], f32, tag="psB")
                    for j in range(NSTREAM):
                        nc.tensor.matmul(
                            out=ps[:, :hw * D],
                            lhsT=ident_sb[:],
                            rhs=gt[:, j * nblk + h: j * nblk + h + hw, :],
                            start=(j == 0), stop=(j == NSTREAM - 1),
                        )
                    msg = bpool.tile([D, 4 * D], f32, tag="msg")
                    nc.scalar.activation(out=msg[:, :hw * D], in_=ps[:, :hw * D],
                                         func=AF.Relu)
                    # k-sum: 64 nodes per graph -> 2 graph rows per block
                    for bi in range(hw):
                        b = h + bi
                        nc.tensor.matmul(
                            out=ps2[:],
                            lhsT=ks_sb[:, b * 16:(b + 1) * 16],
                            rhs=msg[:, bi * D:(bi + 1) * D],
                            start=(b == 0), stop=(b == nblk - 1),
                        )
                # softmax over d for the (up to) 16 graphs of this chunk
                ng = nb // K                       # 16 (last chunk 10)
                s2 = bpool.tile([16, D], f32, tag="s2")
                nc.vector.tensor_copy(out=s2[:ng, :], in_=ps2[:ng, :])
                nmx = bpool.tile([16, 1], f32, tag="nmx")
                nc.vector.tensor_reduce(out=nmx[:ng, :], in_=s2[:ng, :],
                                        axis=mybir.AxisListType.X,
                                        op=ALU.max, negate=True)
                sm = bpool.tile([16, 1], f32, tag="sm")
                ex = bpool.tile([16, D], f32, tag="ex")
                nc.scalar.activation(out=ex[:ng, :], in_=s2[:ng, :], func=AF.Exp,
                                     bias=nmx[:ng, :], accum_out=sm[:ng, :])
                rs = bpool.tile([16, 1], f32, tag="rs")
                nc.vector.reciprocal(out=rs[:ng, :], in_=sm[:ng, :])
                nc.vector.tensor_scalar_mul(out=ex[:ng, :], in0=ex[:ng, :],
                                            scalar1=rs[:ng, :])
                nc.sync.dma_start(out=E_loc_dram[ci * 16:ci * 16 + ng, :],
                                  in_=ex[:ng, :])

        # ---- AllGather E shards ----
        nc.gpsimd.collective_compute(
            "AllGather", ALU.bypass,
            replica_groups=[list(range(NCORES))],
            ins=[E_loc_dram[:].opt()],
            outs=[E_full[:].opt()],
        )

        # =========================== Phase D ===========================
        with tc.tile_pool(name="dpool", bufs=1) as dpool, \
             tc.tile_pool(name="dpsum", bufs=2, space="PSUM") as dppool:
            idx_ext_sb = dpool.tile([D, 256], i16, name="idx_ext_sb")
            nc.sync.dma_start(out=idx_ext_sb[:], in_=idx_ext_in[:])
            gte = dpool.tile([D, 2 * DEXT, D], f32, name="gte")
            for jg in range(4):           # 4 calls of 1024 idxs (4 j's each)
                nc.gpsimd.dma_gather(
                    out_ap=gte[:, jg * 8:(jg + 1) * 8, :],
                    in_ap=E_full[:],
                    idxs_ap=idx_ext_sb[:, jg * 64:(jg + 1) * 64],
                    num_idxs=1024, num_idxs_reg=1024, elem_size=D,
                )
            pse = dppool.tile([D, 2 * D], f32, name="pse")
            for j in range(DEXT):
                nc.tensor.matmul(out=pse[:], lhsT=ident_sb[:],
                                 rhs=gte[:, 2 * j:2 * j + 2, :],
                                 start=(j == 0), stop=(j == DEXT - 1))
            nbrE = dpool.tile([D, 2 * D], f32, name="nbrE")
            nc.scalar.copy(out=nbrE[:], in_=pse[:])

            # local E rows (same data as the shard this core contributed)
            E_loc_sb = dpool.tile([D, 2, D], f32, name="E_loc_sb")
            nc.sync.dma_start(out=E_loc_sb[:, 0, :], in_=E_loc_dram[0:D, :])
            nc.sync.dma_start(out=E_loc_sb[:GL - D, 1, :],
                              in_=E_loc_dram[D:GL, :])

            # transpose E_loc and nbrE -> [d, g]
            ET = dpool.tile([D, 2, D], f32, name="ET")
            NTt = dpool.tile([D, 2, D], f32, name="NTt")
            for rep in range(2):
                pt = dppool.tile([D, D], f32, tag="ptD")
                nc.tensor.transpose(out=pt[:], in_=E_loc_sb[:, rep, :],
                                    identity=ident_sb[:])
                nc.vector.tensor_copy(out=ET[:, rep, :], in_=pt[:])
                pt2 = dppool.tile([D, D], f32, tag="ptD")
                nc.tensor.transpose(out=pt2[:], in_=nbrE[:, rep * D:(rep + 1) * D],
                                    identity=ident_sb[:])
                nc.vector.tensor_copy(out=NTt[:, rep, :], in_=pt2[:])

            UT_sb = dpool.tile([D, D], f32, name="UT_sb")
            nc.sync.dma_start(out=UT_sb[:], in_=UT[:])
            VT_sb = dpool.tile([D, D], f32, name="VT_sb")
            nc.sync.dma_start(out=VT_sb[:], in_=VT[:])

            extT = dpool.tile([D, 2, D], f32, name="extT")
            for rep in range(2):
                ps3 = dppool.tile([D, D], f32, tag="ps3")
                nc.tensor.matmul(out=ps3[:], lhsT=UT_sb[:], rhs=ET[:, rep, :],
                                 start=True, stop=False)
                nc.tensor.matmul(out=ps3[:], lhsT=VT_sb[:], rhs=NTt[:, rep, :],
                                 start=False, stop=True)
                nc.scalar.activation(out=extT[:, rep, :], in_=ps3[:], func=AF.Relu)

            # transpose back -> [g, d], softmax rows -> X
            Xg = dpool.tile([D, 2, D], f32, name="Xg")
            nmx2 = dpool.tile([D, 1], f32, name="nmx2")
            sm2 = dpool.tile([D, 1], f32, name="sm2")
            rs2 = dpool.tile([D, 1], f32, name="rs2")
            for rep in range(2):
                pt3 = dppool.tile([D, D], f32, tag="ptD")
                nc.tensor.transpose(out=pt3[:], in_=extT[:, rep, :],
                                    identity=ident_sb[:])
                gw = D if rep == 0 else GL - D
                nc.vector.tensor_reduce(out=nmx2[:gw, :], in_=pt3[:gw, :],
                                        axis=mybir.AxisListType.X,
                                        op=ALU.max, negate=True)
                nc.scalar.activation(out=Xg[:gw, rep, :], in_=pt3[:gw, :],
                                     func=AF.Exp, bias=nmx2[:gw, :],
                                     accum_out=sm2[:gw, :])
                nc.vector.reciprocal(out=rs2[:gw, :], in_=sm2[:gw, :])
                nc.vector.tensor_scalar_mul(out=Xg[:gw, rep, :],
                                            in0=Xg[:gw, rep, :],
                                            scalar1=rs2[:gw, :])
            nc.sync.dma_start(out=X_loc_dram[0:D, :], in_=Xg[:, 0, :])
            nc.sync.dma_start(out=X_loc_dram[D:GL, :], in_=Xg[:GL - D, 1, :])

        # ---- AllGather X shards ----
        nc.gpsimd.collective_compute(
            "AllGather", ALU.bypass,
            replica_groups=[list(range(NCORES))],
            ins=[X_loc_dram[:].opt()],
            outs=[X_full[:].opt()],
        )

        # =========================== Phase E ===========================
        with tc.tile_pool(name="epool", bufs=1) as epool, \
             tc.tile_pool(name="epsum", bufs=2, space="PSUM") as eppool:
            idx_pair_sb = epool.tile([D, 16], i16, name="idx_pair_sb")
            nc.sync.dma_start(out=idx_pair_sb[:], in_=idx_pair_in[:])
            gtp = epool.tile([D, 2, D], f32, name="gtp")
            nc.gpsimd.dma_gather(
                out_ap=gtp[:], in_ap=X_full[:], idxs_ap=idx_pair_sb[:],
                num_idxs=256, num_idxs_reg=256, elem_size=D,
            )
            m = epool.tile([D, D], f32, name="m")
            nc.vector.tensor_mul(out=m[:], in0=gtp[:, 0, :], in1=gtp[:, 1, :])
            s = epool.tile([D, D], f32, name="s")
            nc.vector.tensor_add(out=s[:], in0=gtp[:, 0, :], in1=gtp[:, 1, :])

            mT = epool.tile([D, D], f32, name="mT")
            sT = epool.tile([D, D], f32, name="sT")
            for src, dst in ((m, mT), (s, sT)):
                ptE = eppool.tile([D, D], f32, tag="ptE")
                nc.tensor.transpose(out=ptE[:], in_=src[:], identity=ident_sb[:])
                nc.vector.tensor_copy(out=dst[:], in_=ptE[:])

            W1mT_sb = epool.tile([D, D], f32, name="W1mT_sb")
            nc.sync.dma_start(out=W1mT_sb[:], in_=W1mT[:])
            W1sT_sb = epool.tile([D, D], f32, name="W1sT_sb")
            nc.sync.dma_start(out=W1sT_sb[:], in_=W1sT[:])
            W2T_sb = epool.tile([D, 2], f32, name="W2T_sb")
            nc.sync.dma_start(out=W2T_sb[:], in_=W2T[:])
            b1_sb = epool.tile([D, 1], f32, name="b1_sb")
            nc.sync.dma_start(out=b1_sb[:], in_=b1_in[:])
            b2_sb = epool.tile([2, 1], f32, name="b2_sb")
            nc.sync.dma_start(out=b2_sb[:], in_=b2_in[:])

            ps4 = eppool.tile([D, D], f32, name="ps4")
            nc.tensor.matmul(out=ps4[:], lhsT=W1mT_sb[:], rhs=mT[:],
                             start=True, stop=False)
            nc.tensor.matmul(out=ps4[:], lhsT=W1sT_sb[:], rhs=sT[:],
                             start=False, stop=True)
            hT = epool.tile([D, D], f32, name="hT")
            nc.scalar.activation(out=hT[:], in_=ps4[:], func=AF.Relu,
                                 bias=b1_sb[:])

            ps5 = eppool.tile([2, D], f32, name="ps5")
            nc.tensor.matmul(out=ps5[:], lhsT=W2T_sb[:], rhs=hT[:],
                             start=True, stop=True)
            lgT = epool.tile([2, D], f32, name="lgT")
            nc.vector.tensor_scalar_add(out=lgT[:], in0=ps5[:], scalar1=b2_sb[:])

            ps6 = eppool.tile([D, 2], f32, name="ps6")
            nc.tensor.transpose(out=ps6[:], in_=lgT[:], identity=ident_sb[:2, :2])
            lg = epool.tile([D, 2], f32, name="lg")
            nc.vector.tensor_copy(out=lg[:], in_=ps6[:])

            nmx3 = epool.tile([D, 1], f32, name="nmx3")
            nc.vector.tensor_reduce(out=nmx3[:], in_=lg[:],
                                    axis=mybir.AxisListType.X,
                                    op=ALU.max, negate=True)
            ex3 = epool.tile([D, 2], f32, name="ex3")
            sm3 = epool.tile([D, 1], f32, name="sm3")
            nc.scalar.activation(out=ex3[:], in_=lg[:], func=AF.Exp,
                                 bias=nmx3[:], accum_out=sm3[:])
            rs3 = epool.tile([D, 1], f32, name="rs3")
            nc.vector.reciprocal(out=rs3[:], in_=sm3[:])
            nc.vector.tensor_scalar_mul(out=ex3[:], in0=ex3[:], scalar1=rs3[:])
            nc.sync.dma_start(out=out_dram[:], in_=ex3[:])

        cpool_cm.__exit__(None, None, None)
        for f in (_f0m, _f0w, _f1, _f2, _f3, _f4):
            f()

    nc.compile()
    return nc


def _prep_in_maps(inputs):
    batch = np.asarray(inputs["batch"])
    node_type = np.asarray(inputs["node_type"])
    nbr_type = np.asarray(inputs["nbr_type"])
    ext_nbr = np.asarray(inputs["ext_nbr"])
    impact = np.asarray(inputs["impact"], dtype=np.float32)
    W = np.asarray(inputs["W"], dtype=np.float32)
    M = np.asarray(inputs["M"], dtype=np.float32)
    U = np.asarray(inputs["U"], dtype=np.float32)
    V = np.asarray(inputs["V"], dtype=np.float32)
    W1 = np.asarray(inputs["W1"], dtype=np.float32)
    b1 = np.asarray(inputs["b1"], dtype=np.float32)
    W2 = np.asarray(inputs["W2"], dtype=np.float32)
    b2 = np.asarray(inputs["b2"], dtype=np.float32)

    ks = np.zeros((D, 8 * 16), dtype=np.float32)
    for bi in range(8):
        ks[:K, bi * 16 + 2 * bi] = 1.0
        ks[K:, bi * 16 + 2 * bi + 1] = 1.0

    shared = dict(
        impact_T=np.ascontiguousarray(impact.T),
        rhs_MW=np.ascontiguousarray(np.concatenate([M.T, W.T], axis=1)),
        UT=np.ascontiguousarray(U.T),
        VT=np.ascontiguousarray(V.T),
        W1mT=np.ascontiguousarray(W1[:, :D].T),
        W1sT=np.ascontiguousarray(W1[:, D:].T),
        W2T=np.ascontiguousarray(W2.T),
        b1=np.ascontiguousarray(b1.reshape(D, 1)),
        b2=np.ascontiguousarray(b2.reshape(2, 1)),
        ident=np.eye(D, dtype=np.float32),
        ks=ks,
    )

    in_maps = []
    for c in range(NCORES):
        g0 = c * GL
        nbr = nbr_type[g0:g0 + GL].reshape(NKL, DIN).astype(np.int64)
        slf = node_type[g0:g0 + GL].reshape(NKL).astype(np.int64)
        parts = []
        for lo, hi in _chunks():
            blocks = [nbr[lo:hi, j] for j in range(DIN)]
            blocks.append(slf[lo:hi])
            parts.append(np.concatenate(blocks))
        idx_big = _wrap16(np.concatenate(parts))

        ex = np.zeros((DEXT, 256), np.int64)
        ex[:, :GL] = ext_nbr[g0:g0 + GL].T
        idx_ext = _wrap16(ex.reshape(-1))

        pair = np.concatenate([
            batch[c * BL:(c + 1) * BL, 0],
            batch[c * BL:(c + 1) * BL, 1],
        ])
        idx_pair = _wrap16(pair)

        m = dict(shared)
        m["idx_big"] = idx_big
        m["idx_ext"] = idx_ext
        m["idx_pair"] = idx_pair
        in_maps.append(m)
    return in_maps


def kernel(**inputs):
    in_maps = _prep_in_maps(inputs)
    if "nc" not in _PROGRAM_CACHE:
        _PROGRAM_CACHE["nc"] = build_program()
    nc = _PROGRAM_CACHE["nc"]

    from concourse import bass_utils
    res = bass_utils.run_bass_kernel_spmd(nc, in_maps, core_ids=list(range(NCORES)))
    out = np.concatenate([r["out"] for r in res.results], axis=0)
    return out.astype(np.float32)



# revision 3
# speedup vs baseline: 2.3551x; 2.3551x over previous
# Trainium2 Bass kernel for DCNNv2 GNN message passing.
#
# Strategy (per spec sharding hint): shard the G (graph) axis data-parallel
# across 8 cores; replicate the 10000x128 impact table and the small weights.
# On each core:
#   Phase A: T2 = [impact @ M.T ; impact @ W.T]  (20000x128 bf16, local DRAM)
#   Phase B: per chunk of 1024 (g,k) nodes, 9 dma_gather streams (8 neighbor
#            rows + 1 self row at offset 10000) from T2, round-robin across
#            the 4 SWDGE queues so descriptor generation runs on all 8 Q7
#            cores in parallel (queue q -> Q7 pair 2q/2q+1).  PE identity-
#            matmul accumulates the 9 rows per node into PSUM, ACT applies
#            relu, PE ones-matmul sums the 64 nodes per graph, softmax -> E.
#   AllGather E -> E_full (Shared DRAM)
#   Phase D: gather E_full[ext_nbr], PE-reduce, U/V matmuls, relu, softmax.
#   AllGather X -> X_full
#   Phase E: link prediction on a 128-pair shard of the batch.
# Host side only marshals data (sharding, int16 index packing, transposes of
# weight matrices); all FLOPs happen on device.

import numpy as np

D = 128
NT = 10000       # impact rows
G = 2000
K = 64
DIN = 8
DEXT = 16
B = 1024
NCORES = 8
GL = G // NCORES           # 250 graphs per core
NKL = GL * K               # 16000 (g,k) rows per core
CHUNK = 1024               # gk rows per gather chunk
NSTREAM = DIN + 1          # 8 neighbor slots + self
BL = B // NCORES           # 128 batch pairs per core
NQ = 4                     # SWDGE queues

_PROGRAM_CACHE = {}


def _chunks():
    out = []
    lo = 0
    while lo < NKL:
        hi = min(lo + CHUNK, NKL)
        out.append((lo, hi))
        lo = hi
    return out


def _idx_cols(n):
    return n * NSTREAM // 16   # int16 idx columns for n gk rows


def _wrap16(flat_i16):
    """Pack a flat int16 index stream for dma_gather: element i at
    [i % 16, i // 16], replicated across the 8 groups of 16 partitions."""
    a = np.asarray(flat_i16, dtype=np.int16).reshape(-1, 16).T   # [16, n/16]
    return np.ascontiguousarray(np.tile(a, (8, 1)))              # [128, n/16]


def build_program():
    import concourse.bacc as bacc
    import concourse.tile as tile
    import concourse.mybir as mybir

    f32 = mybir.dt.float32
    bf16 = mybir.dt.bfloat16
    i16 = mybir.dt.int16
    AF = mybir.ActivationFunctionType
    ALU = mybir.AluOpType

    nc = bacc.Bacc(
        "TRN2",
        target_bir_lowering=False,
        debug=False,
        enable_asserts=False,
        num_devices=NCORES,
        num_swdge_queues=NQ,
    )

    # ---- external inputs (per core) ----
    impact_T = nc.dram_tensor("impact_T", [D, NT], f32, kind="ExternalInput").ap()
    rhs_MW = nc.dram_tensor("rhs_MW", [D, 2 * D], f32, kind="ExternalInput").ap()
    UT = nc.dram_tensor("UT", [D, D], f32, kind="ExternalInput").ap()
    VT = nc.dram_tensor("VT", [D, D], f32, kind="ExternalInput").ap()
    W1mT = nc.dram_tensor("W1mT", [D, D], f32, kind="ExternalInput").ap()
    W1sT = nc.dram_tensor("W1sT", [D, D], f32, kind="ExternalInput").ap()
    W2T = nc.dram_tensor("W2T", [D, 2], f32, kind="ExternalInput").ap()
    b1_in = nc.dram_tensor("b1", [D, 1], f32, kind="ExternalInput").ap()
    b2_in = nc.dram_tensor("b2", [2, 1], f32, kind="ExternalInput").ap()
    ident_in = nc.dram_tensor("ident", [D, D], f32, kind="ExternalInput").ap()
    identh_in = nc.dram_tensor("identh", [D, D], bf16, kind="ExternalInput").ap()
    ks_in = nc.dram_tensor("ks", [D, 8 * 16], f32, kind="ExternalInput").ap()

    n_big_cols = sum(_idx_cols(hi - lo) for lo, hi in _chunks())
    idx_big_in = nc.dram_tensor("idx_big", [D, n_big_cols], i16, kind="ExternalInput").ap()
    idx_ext_in = nc.dram_tensor("idx_ext", [D, 256], i16, kind="ExternalInput").ap()
    idx_pair_in = nc.dram_tensor("idx_pair", [D, 16], i16, kind="ExternalInput").ap()

    out_dram = nc.dram_tensor("out", [BL, 2], f32, kind="ExternalOutput").ap()

    with tile.TileContext(nc) as tc:
        # ---- long-lived DRAM scratch ----
        T2_dram, _f0 = tc.tile([2 * NT, D], bf16, space="DRAM", name="T2_table")
        E_loc_dram, _f1 = tc.tile([GL, D], f32, space="DRAM", name="E_loc")
        E_full, _f2 = tc.tile([G, D], f32, space="DRAM", addr_space="Shared",
                              name="E_full")
        X_loc_dram, _f3 = tc.tile([GL, D], f32, space="DRAM", name="X_loc")
        X_full, _f4 = tc.tile([G, D], f32, space="DRAM", addr_space="Shared",
                              name="X_full")

        # ---- long-lived SBUF constants ----
        cpool_cm = tc.tile_pool(name="consts", bufs=1)
        cpool = cpool_cm.__enter__()
        ident_sb = cpool.tile([D, D], f32, name="ident_sb")
        nc.sync.dma_start(out=ident_sb[:], in_=ident_in[:])
        identh_sb = cpool.tile([D, D], bf16, name="identh_sb")
        nc.sync.dma_start(out=identh_sb[:], in_=identh_in[:])
        ks_sb = cpool.tile([D, 8 * 16], f32, name="ks_sb")
        nc.sync.dma_start(out=ks_sb[:], in_=ks_in[:])
        idx_big_sb = cpool.tile([D, n_big_cols], i16, name="idx_big_sb")
        nc.sync.dma_start(out=idx_big_sb[:], in_=idx_big_in[:])

        # =========================== Phase A ===========================
        # T2[t] = impact[t] @ M.T (t < NT);  T2[NT+t] = impact[t] @ W.T
        with tc.tile_pool(name="phaseA_sb", bufs=3) as apool, \
             tc.tile_pool(name="phaseA_ps", bufs=4, space="PSUM") as appool, \
             tc.tile_pool(name="phaseA_imp", bufs=3) as ipool:
            mw_sb = apool.tile([D, 2 * D], f32, name="mw_sb")
            nc.sync.dma_start(out=mw_sb[:], in_=rhs_MW[:])

            n_tiles = (NT + D - 1) // D        # 79
            GRP = 8
            t = 0
            while t < n_tiles:
                ns = min(GRP, n_tiles - t)
                gw = min(ns * D, NT - t * D)
                imp_g = ipool.tile([D, GRP * D], f32, tag="impg")
                nc.sync.dma_start(out=imp_g[:, :gw],
                                  in_=impact_T[:, t * D:t * D + gw])
                stage = apool.tile([D, ns, 2 * D], bf16, tag="stageA")
                for s in range(ns):
                    tw = min(D, NT - (t + s) * D)      # 128, last tile 16
                    psA = appool.tile([D, 2 * D], f32, tag="psA")
                    nc.tensor.matmul(
                        out=psA[:tw, :],
                        lhsT=imp_g[:, s * D:s * D + tw],
                        rhs=mw_sb[:],
                        start=True, stop=True,
                    )
                    if s % 2 == 0:
                        nc.scalar.copy(out=stage[:tw, s, :], in_=psA[:tw, :])
                    else:
                        nc.vector.tensor_copy(out=stage[:tw, s, :], in_=psA[:tw, :])
                # full 128-row tiles in this group
                nf = ns if (t + ns) * D <= NT else ns - 1
                base = t * D
                if nf > 0:
                    nrows = nf * D
                    nc.sync.dma_start(
                        out=T2_dram[base:base + nrows, :]
                            .rearrange("(s p) d -> p s d", p=D),
                        in_=stage[:, :nf, 0:D],
                    )
                    nc.sync.dma_start(
                        out=T2_dram[NT + base:NT + base + nrows, :]
                            .rearrange("(s p) d -> p s d", p=D),
                        in_=stage[:, :nf, D:2 * D],
                    )
                if nf < ns:  # partial last tile (16 rows)
                    pb = base + nf * D
                    pw = NT - pb
                    nc.sync.dma_start(out=T2_dram[pb:pb + pw, :],
                                      in_=stage[:pw, nf, 0:D])
                    nc.sync.dma_start(out=T2_dram[NT + pb:NT + pb + pw, :],
                                      in_=stage[:pw, nf, D:2 * D])
                t += ns

        # =========================== Phase B ===========================
        gq = 0   # global gather counter for queue round-robin
        with tc.tile_pool(name="gpool", bufs=3) as gpool, \
             tc.tile_pool(name="bpool", bufs=3) as bpool, \
             tc.tile_pool(name="bpsum", bufs=3, space="PSUM") as bppool, \
             tc.tile_pool(name="b2psum", bufs=2, space="PSUM") as b2ppool:
            col0 = 0
            for ci, (lo, hi) in enumerate(_chunks()):
                nb = hi - lo
                nblk = nb // D                     # 8 (last chunk 5)
                ncols = _idx_cols(nb)
                gt = gpool.tile([D, NSTREAM * nblk, D], bf16, tag="gt")
                jcols = nb // 16          # idx cols per j-stream (<= 64)
                for j in range(NSTREAM):
                    nc.gpsimd.dma_gather(
                        out_ap=gt[:, j * nblk:(j + 1) * nblk, :],
                        in_ap=T2_dram[:],
                        idxs_ap=idx_big_sb[:, col0 + j * jcols:
                                           col0 + (j + 1) * jcols],
                        num_idxs=nb,
                        num_idxs_reg=nb,
                        elem_size=D,
                        queue_num=gq % NQ,
                    )
                    gq += 1
                col0 += ncols

                ps2 = b2ppool.tile([16, D], f32, tag="ps2")
                for h in range(0, nblk, 4):
                    hw = min(4, nblk - h)
                    ps = bppool.tile([D, 4 * D], f32, tag="psB")
                    for j in range(NSTREAM):
                        nc.tensor.matmul(
                            out=ps[:, :hw * D],
                            lhsT=identh_sb[:],
                            rhs=gt[:, j * nblk + h: j * nblk + h + hw, :],
                            start=(j == 0), stop=(j == NSTREAM - 1),
                        )
                    msg = bpool.tile([D, 4 * D], f32, tag="msg")
                    nc.scalar.activation(out=msg[:, :hw * D], in_=ps[:, :hw * D],
                                         func=AF.Relu)
                    # k-sum: 64 nodes per graph -> 2 graph rows per block
                    for bi in range(hw):
                        b = h + bi
                        nc.tensor.matmul(
                            out=ps2[:],
                            lhsT=ks_sb[:, b * 16:(b + 1) * 16],
                            rhs=msg[:, bi * D:(bi + 1) * D],
                            start=(b == 0), stop=(b == nblk - 1),
                        )
                # softmax over d for the (up to) 16 graphs of this chunk
                ng = nb // K                       # 16 (last chunk 10)
                s2 = bpool.tile([16, D], f32, tag="s2")
                nc.vector.tensor_copy(out=s2[:ng, :], in_=ps2[:ng, :])
                nmx = bpool.tile([16, 1], f32, tag="nmx")
                nc.vector.tensor_reduce(out=nmx[:ng, :], in_=s2[:ng, :],
                                        axis=mybir.AxisListType.X,
                                        op=ALU.max, negate=True)
                sm = bpool.tile([16, 1], f32, tag="sm")
                ex = bpool.tile([16, D], f32, tag="ex")
                nc.scalar.activation(out=ex[:ng, :], in_=s2[:ng, :], func=AF.Exp,
                                     bias=nmx[:ng, :], accum_out=sm[:ng, :])
                rs = bpool.tile([16, 1], f32, tag="rs")
                nc.vector.reciprocal(out=rs[:ng, :], in_=sm[:ng, :])
                nc.vector.tensor_scalar_mul(out=ex[:ng, :], in0=ex[:ng, :],
                                            scalar1=rs[:ng, :])
                nc.sync.dma_start(out=E_loc_dram[ci * 16:ci * 16 + ng, :],
                                  in_=ex[:ng, :])

        # ---- AllGather E shards ----
        nc.gpsimd.collective_compute(
            "AllGather", ALU.bypass,
            replica_groups=[list(range(NCORES))],
            ins=[E_loc_dram[:].opt()],
            outs=[E_full[:].opt()],
        )

        # =========================== Phase D ===========================
        with tc.tile_pool(name="dpool", bufs=1) as dpool, \
             tc.tile_pool(name="dpsum", bufs=2, space="PSUM") as dppool:
            idx_ext_sb = dpool.tile([D, 256], i16, name="idx_ext_sb")
            nc.sync.dma_start(out=idx_ext_sb[:], in_=idx_ext_in[:])
            gte = dpool.tile([D, 2 * DEXT, D], f32, name="gte")
            for jg in range(4):           # 4 calls of 1024 idxs (4 j's each)
                nc.gpsimd.dma_gather(
                    out_ap=gte[:, jg * 8:(jg + 1) * 8, :],
                    in_ap=E_full[:],
                    idxs_ap=idx_ext_sb[:, jg * 64:(jg + 1) * 64],
                    num_idxs=1024, num_idxs_reg=1024, elem_size=D,
                    queue_num=jg % NQ,
                )
            pse = dppool.tile([D, 2 * D], f32, name="pse")
            for j in range(DEXT):
                nc.tensor.matmul(out=pse[:], lhsT=ident_sb[:],
                                 rhs=gte[:, 2 * j:2 * j + 2, :],
                                 start=(j == 0), stop=(j == DEXT - 1))
            nbrE = dpool.tile([D, 2 * D], f32, name="nbrE")
            nc.scalar.copy(out=nbrE[:], in_=pse[:])

            # local E rows (same data as the shard this core contributed)
            E_loc_sb = dpool.tile([D, 2, D], f32, name="E_loc_sb")
            nc.sync.dma_start(out=E_loc_sb[:, 0, :], in_=E_loc_dram[0:D, :])
            nc.sync.dma_start(out=E_loc_sb[:GL - D, 1, :],
                              in_=E_loc_dram[D:GL, :])

            # transpose E_loc and nbrE -> [d, g]
            ET = dpool.tile([D, 2, D], f32, name="ET")
            NTt = dpool.tile([D, 2, D], f32, name="NTt")
            for rep in range(2):
                pt = dppool.tile([D, D], f32, tag="ptD")
                nc.tensor.transpose(out=pt[:], in_=E_loc_sb[:, rep, :],
                                    identity=ident_sb[:])
                nc.vector.tensor_copy(out=ET[:, rep, :], in_=pt[:])
                pt2 = dppool.tile([D, D], f32, tag="ptD")
                nc.tensor.transpose(out=pt2[:], in_=nbrE[:, rep * D:(rep + 1) * D],
                                    identity=ident_sb[:])
                nc.vector.tensor_copy(out=NTt[:, rep, :], in_=pt2[:])

            UT_sb = dpool.tile([D, D], f32, name="UT_sb")
            nc.sync.dma_start(out=UT_sb[:], in_=UT[:])
            VT_sb = dpool.tile([D, D], f32, name="VT_sb")
            nc.sync.dma_start(out=VT_sb[:], in_=VT[:])

            extT = dpool.tile([D, 2, D], f32, name="extT")
            for rep in range(2):
                ps3 = dppool.tile([D, D], f32, tag="ps3")
                nc.tensor.matmul(out=ps3[:], lhsT=UT_sb[:], rhs=ET[:, rep, :],
                                 start=True, stop=False)
                nc.tensor.matmul(out=ps3[:], lhsT=VT_sb[:], rhs=NTt[:, rep, :],
                                 start=False, stop=True)
                nc.scalar.activation(out=extT[:, rep, :], in_=ps3[:], func=AF.Relu)

            # transpose back -> [g, d], softmax rows -> X
            Xg = dpool.tile([D, 2, D], f32, name="Xg")
            nmx2 = dpool.tile([D, 1], f32, name="nmx2")
            sm2 = dpool.tile([D, 1], f32, name="sm2")
            rs2 = dpool.tile([D, 1], f32, name="rs2")
            for rep in range(2):
                pt3 = dppool.tile([D, D], f32, tag="ptD")
                nc.tensor.transpose(out=pt3[:], in_=extT[:, rep, :],
                                    identity=ident_sb[:])
                gw = D if rep == 0 else GL - D
                nc.vector.tensor_reduce(out=nmx2[:gw, :], in_=pt3[:gw, :],
                                        axis=mybir.AxisListType.X,
                                        op=ALU.max, negate=True)
                nc.scalar.activation(out=Xg[:gw, rep, :], in_=pt3[:gw, :],
                                     func=AF.Exp, bias=nmx2[:gw, :],
                                     accum_out=sm2[:gw, :])
                nc.vector.reciprocal(out=rs2[:gw, :], in_=sm2[:gw, :])
                nc.vector.tensor_scalar_mul(out=Xg[:gw, rep, :],
                                            in0=Xg[:gw, rep, :],
                                            scalar1=rs2[:gw, :])
            nc.sync.dma_start(out=X_loc_dram[0:D, :], in_=Xg[:, 0, :])
            nc.sync.dma_start(out=X_loc_dram[D:GL, :], in_=Xg[:GL - D, 1, :])

        # ---- AllGather X shards ----
        nc.gpsimd.collective_compute(
            "AllGather", ALU.bypass,
            replica_groups=[list(range(NCORES))],
            ins=[X_loc_dram[:].opt()],
            outs=[X_full[:].opt()],
        )

        # =========================== Phase E ===========================
        with tc.tile_pool(name="epool", bufs=1) as epool, \
             tc.tile_pool(name="epsum", bufs=2, space="PSUM") as eppool:
            idx_pair_sb = epool.tile([D, 16], i16, name="idx_pair_sb")
            nc.sync.dma_start(out=idx_pair_sb[:], in_=idx_pair_in[:])
            gtp = epool.tile([D, 2, D], f32, name="gtp")
            nc.gpsimd.dma_gather(
                out_ap=gtp[:], in_ap=X_full[:], idxs_ap=idx_pair_sb[:],
                num_idxs=256, num_idxs_reg=256, elem_size=D,
            )
            m = epool.tile([D, D], f32, name="m")
            nc.vector.tensor_mul(out=m[:], in0=gtp[:, 0, :], in1=gtp[:, 1, :])
            s = epool.tile([D, D], f32, name="s")
            nc.vector.tensor_add(out=s[:], in0=gtp[:, 0, :], in1=gtp[:, 1, :])

            mT = epool.tile([D, D], f32, name="mT")
            sT = epool.tile([D, D], f32, name="sT")
            for src, dst in ((m, mT), (s, sT)):
                ptE = eppool.tile([D, D], f32, tag="ptE")
                nc.tensor.transpose(out=ptE[:], in_=src[:], identity=ident_sb[:])
                nc.vector.tensor_copy(out=dst[:], in_=ptE[:])

            W1mT_sb = epool.tile([D, D], f32, name="W1mT_sb")
            nc.sync.dma_start(out=W1mT_sb[:], in_=W1mT[:])
            W1sT_sb = epool.tile([D, D], f32, name="W1sT_sb")
            nc.sync.dma_start(out=W1sT_sb[:], in_=W1sT[:])
            W2T_sb = epool.tile([D, 2], f32, name="W2T_sb")
            nc.sync.dma_start(out=W2T_sb[:], in_=W2T[:])
            b1_sb = epool.tile([D, 1], f32, name="b1_sb")
            nc.sync.dma_start(out=b1_sb[:], in_=b1_in[:])
            b2_sb = epool.tile([2, 1], f32, name="b2_sb")
            nc.sync.dma_start(out=b2_sb[:], in_=b2_in[:])

            ps4 = eppool.tile([D, D], f32, name="ps4")
            nc.tensor.matmul(out=ps4[:], lhsT=W1mT_sb[:], rhs=mT[:],
                             start=True, stop=False)
            nc.tensor.matmul(out=ps4[:], lhsT=W1sT_sb[:], rhs=sT[:],
                             start=False, stop=True)
            hT = epool.tile([D, D], f32, name="hT")
            nc.scalar.activation(out=hT[:], in_=ps4[:], func=AF.Relu,
                                 bias=b1_sb[:])

            ps5 = eppool.tile([2, D], f32, name="ps5")
            nc.tensor.matmul(out=ps5[:], lhsT=W2T_sb[:], rhs=hT[:],
                             start=True, stop=True)
            lgT = epool.tile([2, D], f32, name="lgT")
            nc.vector.tensor_scalar_add(out=lgT[:], in0=ps5[:], scalar1=b2_sb[:])

            ps6 = eppool.tile([D, 2], f32, name="ps6")
            nc.tensor.transpose(out=ps6[:], in_=lgT[:], identity=ident_sb[:2, :2])
            lg = epool.tile([D, 2], f32, name="lg")
            nc.vector.tensor_copy(out=lg[:], in_=ps6[:])

            nmx3 = epool.tile([D, 1], f32, name="nmx3")
            nc.vector.tensor_reduce(out=nmx3[:], in_=lg[:],
                                    axis=mybir.AxisListType.X,
                                    op=ALU.max, negate=True)
            ex3 = epool.tile([D, 2], f32, name="ex3")
            sm3 = epool.tile([D, 1], f32, name="sm3")
            nc.scalar.activation(out=ex3[:], in_=lg[:], func=AF.Exp,
                                 bias=nmx3[:], accum_out=sm3[:])
            rs3 = epool.tile([D, 1], f32, name="rs3")
            nc.vector.reciprocal(out=rs3[:], in_=sm3[:])
            nc.vector.tensor_scalar_mul(out=ex3[:], in0=ex3[:], scalar1=rs3[:])
            nc.sync.dma_start(out=out_dram[:], in_=ex3[:])

        cpool_cm.__exit__(None, None, None)
        for f in (_f0, _f1, _f2, _f3, _f4):
            f()

    nc.compile()
    return nc


def _prep_in_maps(inputs):
    batch = np.asarray(inputs["batch"])
    node_type = np.asarray(inputs["node_type"])
    nbr_type = np.asarray(inputs["nbr_type"])
    ext_nbr = np.asarray(inputs["ext_nbr"])
    impact = np.asarray(inputs["impact"], dtype=np.float32)
    W = np.asarray(inputs["W"], dtype=np.float32)
    M = np.asarray(inputs["M"], dtype=np.float32)
    U = np.asarray(inputs["U"], dtype=np.float32)
    V = np.asarray(inputs["V"], dtype=np.float32)
    W1 = np.asarray(inputs["W1"], dtype=np.float32)
    b1 = np.asarray(inputs["b1"], dtype=np.float32)
    W2 = np.asarray(inputs["W2"], dtype=np.float32)
    b2 = np.asarray(inputs["b2"], dtype=np.float32)

    ks = np.zeros((D, 8 * 16), dtype=np.float32)
    for bi in range(8):
        ks[:K, bi * 16 + 2 * bi] = 1.0
        ks[K:, bi * 16 + 2 * bi + 1] = 1.0

    import ml_dtypes
    ident = np.eye(D, dtype=np.float32)
    identh = ident.astype(ml_dtypes.bfloat16)
    shared = dict(
        impact_T=np.ascontiguousarray(impact.T),
        rhs_MW=np.ascontiguousarray(np.concatenate([M.T, W.T], axis=1)),
        UT=np.ascontiguousarray(U.T),
        VT=np.ascontiguousarray(V.T),
        W1mT=np.ascontiguousarray(W1[:, :D].T),
        W1sT=np.ascontiguousarray(W1[:, D:].T),
        W2T=np.ascontiguousarray(W2.T),
        b1=np.ascontiguousarray(b1.reshape(D, 1)),
        b2=np.ascontiguousarray(b2.reshape(2, 1)),
        ident=ident,
        identh=np.ascontiguousarray(identh),
        ks=ks,
    )

    in_maps = []
    for c in range(NCORES):
        g0 = c * GL
        nbr = nbr_type[g0:g0 + GL].reshape(NKL, DIN).astype(np.int64)
        slf = node_type[g0:g0 + GL].reshape(NKL).astype(np.int64)
        parts = []
        for lo, hi in _chunks():
            blocks = [nbr[lo:hi, j] for j in range(DIN)]
            blocks.append(NT + slf[lo:hi])
            parts.append(np.concatenate(blocks))
        idx_big = _wrap16(np.concatenate(parts))

        ex = np.zeros((DEXT, 256), np.int64)
        ex[:, :GL] = ext_nbr[g0:g0 + GL].T
        idx_ext = _wrap16(ex.reshape(-1))

        pair = np.concatenate([
            batch[c * BL:(c + 1) * BL, 0],
            batch[c * BL:(c + 1) * BL, 1],
        ])
        idx_pair = _wrap16(pair)

        m = dict(shared)
        m["idx_big"] = idx_big
        m["idx_ext"] = idx_ext
        m["idx_pair"] = idx_pair
        in_maps.append(m)
    return in_maps


def kernel(**inputs):
    in_maps = _prep_in_maps(inputs)
    if "nc" not in _PROGRAM_CACHE:
        _PROGRAM_CACHE["nc"] = build_program()
    nc = _PROGRAM_CACHE["nc"]

    from concourse import bass_utils
    res = bass_utils.run_bass_kernel_spmd(nc, in_maps, core_ids=list(range(NCORES)))
    out = np.concatenate([r["out"] for r in res.results], axis=0)
    return out.astype(np.float32)


# revision 14
# speedup vs baseline: 2.9934x; 1.2710x over previous
# Trainium2 Bass kernel for DCNNv2 GNN message passing.
#
# Strategy (per spec sharding hint): shard the G (graph) axis data-parallel
# across 8 cores; replicate the 10000x128 impact table and the small weights.
# On each core:
#   Phase A: T2 = [impact @ M.T ; impact @ W.T]  (20000x128 bf16, local DRAM)
#   Phase B: per chunk of 1024 (g,k) nodes, 9 dma_gather streams (8 neighbor
#            rows + 1 self row at offset 10000) from T2, round-robin across
#            the 4 SWDGE queues so descriptor generation runs on all 8 Q7
#            cores in parallel (queue q -> Q7 pair 2q/2q+1).  PE identity-
#            matmul accumulates the 9 rows per node into PSUM, ACT applies
#            relu, PE ones-matmul sums the 64 nodes per graph, softmax -> E.
#   AllGather E -> E_full (Shared DRAM)
#   Phase D: gather E_full[ext_nbr], PE-reduce, U/V matmuls, relu, softmax.
#   AllGather X -> X_full
#   Phase E: link prediction on a 128-pair shard of the batch.
# Host side only marshals data (sharding, int16 index packing, transposes of
# weight matrices); all FLOPs happen on device.

import numpy as np

D = 128
NT = 10000       # impact rows
G = 2000
K = 64
DIN = 8
DEXT = 16
B = 1024
NCORES = 8
GL = G // NCORES           # 250 graphs per core
NKL = GL * K               # 16000 (g,k) rows per core
CHUNK = 1024               # gk rows per gather chunk
NSTREAM = DIN + 1          # 8 neighbor slots + self
BL = B // NCORES           # 128 batch pairs per core
NQ = 4                     # SWDGE queues

_PROGRAM_CACHE = {}


def _chunks():
    out = []
    lo = 0
    while lo < NKL:
        hi = min(lo + CHUNK, NKL)
        out.append((lo, hi))
        lo = hi
    return out


def _idx_cols(n):
    return n * NSTREAM // 16   # int16 idx columns for n gk rows


def _wrap16(flat_i16):
    """Pack a flat int16 index stream for dma_gather: element i at
    [i % 16, i // 16], replicated across the 8 groups of 16 partitions."""
    a = np.asarray(flat_i16, dtype=np.int16).reshape(-1, 16).T   # [16, n/16]
    return np.ascontiguousarray(np.tile(a, (8, 1)))              # [128, n/16]


def build_program():
    import concourse.bacc as bacc
    import concourse.tile as tile
    import concourse.mybir as mybir

    f32 = mybir.dt.float32
    bf16 = mybir.dt.bfloat16
    i16 = mybir.dt.int16
    AF = mybir.ActivationFunctionType
    ALU = mybir.AluOpType

    nc = bacc.Bacc(
        "TRN2",
        target_bir_lowering=False,
        debug=False,
        enable_asserts=False,
        num_devices=NCORES,
        num_swdge_queues=NQ,
    )

    # ---- external inputs (per core) ----
    impact_T = nc.dram_tensor("impact_T", [D, NT], f32, kind="ExternalInput").ap()
    rhs_MW = nc.dram_tensor("rhs_MW", [D, 2 * D], f32, kind="ExternalInput").ap()
    UT = nc.dram_tensor("UT", [D, D], f32, kind="ExternalInput").ap()
    VT = nc.dram_tensor("VT", [D, D], f32, kind="ExternalInput").ap()
    W1mT = nc.dram_tensor("W1mT", [D, D], f32, kind="ExternalInput").ap()
    W1sT = nc.dram_tensor("W1sT", [D, D], f32, kind="ExternalInput").ap()
    W2T = nc.dram_tensor("W2T", [D, 2], f32, kind="ExternalInput").ap()
    b1_in = nc.dram_tensor("b1", [D, 1], f32, kind="ExternalInput").ap()
    b2_in = nc.dram_tensor("b2", [2, 1], f32, kind="ExternalInput").ap()
    ident_in = nc.dram_tensor("ident", [D, D], f32, kind="ExternalInput").ap()
    identh_in = nc.dram_tensor("identh", [D, D], bf16, kind="ExternalInput").ap()
    ks_in = nc.dram_tensor("ks", [D, 8 * 16], f32, kind="ExternalInput").ap()

    n_big_cols = sum(_idx_cols(hi - lo) for lo, hi in _chunks())
    idx_big_in = nc.dram_tensor("idx_big", [D, n_big_cols], i16, kind="ExternalInput").ap()
    idx_ext_in = nc.dram_tensor("idx_ext", [D, 256], i16, kind="ExternalInput").ap()
    idx_pair_in = nc.dram_tensor("idx_pair", [D, 16], i16, kind="ExternalInput").ap()

    out_dram = nc.dram_tensor("out", [BL, 2], f32, kind="ExternalOutput").ap()

    with tile.TileContext(nc) as tc:
        # ---- long-lived DRAM scratch ----
        T2_dram, _f0 = tc.tile([2 * NT, D], bf16, space="DRAM", name="T2_table")
        E_loc_dram, _f1 = tc.tile([GL, D], bf16, space="DRAM", name="E_loc")
        E_full, _f2 = tc.tile([G, D], bf16, space="DRAM", addr_space="Shared",
                              name="E_full")
        X_loc_dram, _f3 = tc.tile([GL, D], bf16, space="DRAM", name="X_loc")
        X_full, _f4 = tc.tile([G, D], bf16, space="DRAM", addr_space="Shared",
                              name="X_full")

        # ---- long-lived SBUF constants ----
        cpool_cm = tc.tile_pool(name="consts", bufs=1)
        cpool = cpool_cm.__enter__()
        ident_sb = cpool.tile([D, D], f32, name="ident_sb")
        nc.sync.dma_start(out=ident_sb[:], in_=ident_in[:])
        identh_sb = cpool.tile([D, D], bf16, name="identh_sb")
        nc.sync.dma_start(out=identh_sb[:], in_=identh_in[:])
        ks_sb = cpool.tile([D, 8 * 16], bf16, name="ks_sb")
        nc.gpsimd.dma_start(out=ks_sb[:], in_=ks_in[:])
        idx_big_sb = cpool.tile([D, n_big_cols], i16, name="idx_big_sb")
        nc.sync.dma_start(out=idx_big_sb[:], in_=idx_big_in[:])

        # =========================== Phase A ===========================
        # T2[t] = impact[t] @ M.T (t < NT);  T2[NT+t] = impact[t] @ W.T
        # impact and [M.T;W.T] are cast to bf16 during the load DMA (SWDGE
        # cast path; the Pool engine is idle during phase A) so the 79
        # matmuls run at bf16 rate.
        with tc.tile_pool(name="phaseA_sb", bufs=3) as apool, \
             tc.tile_pool(name="phaseA_ps", bufs=4, space="PSUM") as appool, \
             tc.tile_pool(name="phaseA_imp", bufs=3) as ipool:
            mw_sb = apool.tile([D, 2 * D], bf16, name="mw_sb")
            nc.gpsimd.dma_start(out=mw_sb[:], in_=rhs_MW[:])

            n_tiles = (NT + D - 1) // D        # 79
            GRP = 8
            t = 0
            while t < n_tiles:
                ns = min(GRP, n_tiles - t)
                gw = min(ns * D, NT - t * D)
                imp_g = ipool.tile([D, GRP * D], bf16, tag="impg")
                nc.gpsimd.dma_start(out=imp_g[:, :gw],
                                    in_=impact_T[:, t * D:t * D + gw])
                stage = apool.tile([D, ns, 2 * D], bf16, tag="stageA")
                for s in range(ns):
                    tw = min(D, NT - (t + s) * D)      # 128, last tile 16
                    psA = appool.tile([D, 2 * D], f32, tag="psA")
                    nc.tensor.matmul(
                        out=psA[:tw, :],
                        lhsT=imp_g[:, s * D:s * D + tw],
                        rhs=mw_sb[:],
                        start=True, stop=True,
                    )
                    if s % 2 == 0:
                        nc.scalar.copy(out=stage[:tw, s, :], in_=psA[:tw, :])
                    else:
                        nc.vector.tensor_copy(out=stage[:tw, s, :], in_=psA[:tw, :])
                # full 128-row tiles in this group
                nf = ns if (t + ns) * D <= NT else ns - 1
                base = t * D
                if nf > 0:
                    nrows = nf * D
                    nc.sync.dma_start(
                        out=T2_dram[base:base + nrows, :]
                            .rearrange("(s p) d -> p s d", p=D),
                        in_=stage[:, :nf, 0:D],
                    )
                    nc.sync.dma_start(
                        out=T2_dram[NT + base:NT + base + nrows, :]
                            .rearrange("(s p) d -> p s d", p=D),
                        in_=stage[:, :nf, D:2 * D],
                    )
                if nf < ns:  # partial last tile (16 rows)
                    pb = base + nf * D
                    pw = NT - pb
                    nc.sync.dma_start(out=T2_dram[pb:pb + pw, :],
                                      in_=stage[:pw, nf, 0:D])
                    nc.sync.dma_start(out=T2_dram[NT + pb:NT + pb + pw, :],
                                      in_=stage[:pw, nf, D:2 * D])
                t += ns

        # =========================== Phase B ===========================
        # per-graph pre-softmax rows accumulate in E_pre ([128, 2, D]:
        # graph r on partition r % 128, rep r // 128); softmax runs once,
        # batched, at the end of the phase.
        E_pre = cpool.tile([D, 2, D], f32, name="E_pre")
        E_out = cpool.tile([D, 2, D], bf16, name="E_out")
        gq = 0   # global gather counter for queue round-robin
        with tc.tile_pool(name="gpool", bufs=4) as gpool, \
             tc.tile_pool(name="bpool", bufs=4) as bpool, \
             tc.tile_pool(name="bpsum", bufs=4, space="PSUM") as bppool, \
             tc.tile_pool(name="b2psum", bufs=2, space="PSUM") as b2ppool:
            col0 = 0
            for ci, (lo, hi) in enumerate(_chunks()):
                nb = hi - lo
                nblk = nb // D                     # 8 (last chunk 5)
                ncols = _idx_cols(nb)
                gt = gpool.tile([D, NSTREAM * nblk, D], bf16, tag="gt")
                jcols = nb // 16          # idx cols per j-stream (<= 64)
                for j in range(NSTREAM):
                    nc.gpsimd.dma_gather(
                        out_ap=gt[:, j * nblk:(j + 1) * nblk, :],
                        in_ap=T2_dram[:],
                        idxs_ap=idx_big_sb[:, col0 + j * jcols:
                                           col0 + (j + 1) * jcols],
                        num_idxs=nb,
                        num_idxs_reg=nb,
                        elem_size=D,
                        queue_num=gq % NQ,
                    )
                    gq += 1
                col0 += ncols

                # all accumulate matmuls first (identity stays stationary),
                # then relu, then the k-sum matmuls.
                pss = []
                for h in range(0, nblk, 4):
                    hw = min(4, nblk - h)
                    ps = bppool.tile([D, 4 * D], f32, tag="psB")
                    for j in range(NSTREAM):
                        nc.tensor.matmul(
                            out=ps[:, :hw * D],
                            lhsT=identh_sb[:],
                            rhs=gt[:, j * nblk + h: j * nblk + h + hw, :],
                            start=(j == 0), stop=(j == NSTREAM - 1),
                        )
                    pss.append((h, hw, ps))
                msgs = []
                for h, hw, ps in pss:
                    msg = bpool.tile([D, 4 * D], bf16, tag="msg")
                    nc.scalar.activation(out=msg[:, :hw * D], in_=ps[:, :hw * D],
                                         func=AF.Relu)
                    msgs.append((h, hw, msg))
                ps2 = b2ppool.tile([16, D], f32, tag="ps2")
                for h, hw, msg in msgs:
                    # k-sum: 64 nodes per graph -> 2 graph rows per block
                    for bi in range(hw):
                        b = h + bi
                        nc.tensor.matmul(
                            out=ps2[:],
                            lhsT=ks_sb[:, b * 16:(b + 1) * 16],
                            rhs=msg[:, bi * D:(bi + 1) * D],
                            start=(b == 0), stop=(b == nblk - 1),
                        )
                ng = nb // K                       # 16 (last chunk 10)
                r0 = ci * 16                       # first graph row of chunk
                s2 = bpool.tile([16, D], f32, tag="s2")
                nc.vector.tensor_copy(out=s2[:ng, :], in_=ps2[:ng, :])
                nc.sync.dma_start(
                    out=E_pre[r0 % D:r0 % D + ng, r0 // D, :],
                    in_=s2[:ng, :])

                # as each half of E_pre completes, softmax it and kick off
                # its AllGather so the first one overlaps the second half
                # of phase B.  E_full rows: graph (c, r) -> c*128 + r for
                # r < 128, else 1024 + c*122 + (r - 128).
                if ci in (7, len(_chunks()) - 1):
                    rep = 0 if ci == 7 else 1
                    gw = D if rep == 0 else GL - D
                    nmx = bpool.tile([D, 1], f32, tag="nmxE")
                    nc.vector.tensor_reduce(out=nmx[:gw, :],
                                            in_=E_pre[:gw, rep, :],
                                            axis=mybir.AxisListType.X,
                                            op=ALU.max, negate=True)
                    sm = bpool.tile([D, 1], f32, tag="smE")
                    ex = bpool.tile([D, D], f32, tag="exE")
                    nc.scalar.activation(out=ex[:gw, :], in_=E_pre[:gw, rep, :],
                                         func=AF.Exp, bias=nmx[:gw, :],
                                         accum_out=sm[:gw, :])
                    rs = bpool.tile([D, 1], f32, tag="rsE")
                    nc.vector.reciprocal(out=rs[:gw, :], in_=sm[:gw, :])
                    nc.vector.tensor_scalar_mul(out=E_out[:gw, rep, :],
                                                in0=ex[:gw, :],
                                                scalar1=rs[:gw, :])
                    nc.sync.dma_start(out=E_loc_dram[rep * D:rep * D + gw, :],
                                      in_=E_out[:gw, rep, :])
                    nc.gpsimd.collective_compute(
                        "AllGather", ALU.bypass,
                        replica_groups=[list(range(NCORES))],
                        ins=[E_loc_dram[rep * D:rep * D + gw, :].opt()],
                        outs=[E_full[rep * NCORES * D:
                                     rep * NCORES * D + NCORES * gw, :].opt()],
                    )

        # =========================== Phase D ===========================
        with tc.tile_pool(name="dpool", bufs=1) as dpool, \
             tc.tile_pool(name="dpsum", bufs=2, space="PSUM") as dppool:
            UT_sb = dpool.tile([D, D], f32, name="UT_sb")
            nc.sync.dma_start(out=UT_sb[:], in_=UT[:])
            VT_sb = dpool.tile([D, D], f32, name="VT_sb")
            nc.sync.dma_start(out=VT_sb[:], in_=VT[:])
            idx_ext_sb = dpool.tile([D, 256], i16, name="idx_ext_sb")
            nc.sync.dma_start(out=idx_ext_sb[:], in_=idx_ext_in[:])

            # U-part primes early: transpose local E (in SBUF since phase B)
            # and start the U matmuls; they only depend on E_out, not on the
            # AllGathers.
            ET = dpool.tile([D, 2, D], f32, name="ET")
            ps3s = []
            for rep in range(2):
                pt = dppool.tile([D, D], bf16, tag="ptDh")
                nc.tensor.transpose(out=pt[:], in_=E_out[:, rep, :],
                                    identity=identh_sb[:])
                nc.vector.tensor_copy(out=ET[:, rep, :], in_=pt[:])
                ps3 = dppool.tile([D, D], f32, tag="ps3")
                nc.tensor.matmul(out=ps3[:], lhsT=UT_sb[:], rhs=ET[:, rep, :],
                                 start=True, stop=False)
                ps3s.append(ps3)

            gte = dpool.tile([D, 2 * DEXT, D], bf16, name="gte")
            for jg in range(4):           # 4 calls of 1024 idxs (4 j's each)
                nc.gpsimd.dma_gather(
                    out_ap=gte[:, jg * 8:(jg + 1) * 8, :],
                    in_ap=E_full[:],
                    idxs_ap=idx_ext_sb[:, jg * 64:(jg + 1) * 64],
                    num_idxs=1024, num_idxs_reg=1024, elem_size=D,
                    queue_num=jg % NQ,
                )
            pse = dppool.tile([D, 2 * D], f32, name="pse")
            for j in range(DEXT):
                nc.tensor.matmul(out=pse[:], lhsT=identh_sb[:],
                                 rhs=gte[:, 2 * j:2 * j + 2, :],
                                 start=(j == 0), stop=(j == DEXT - 1))
            nbrE = dpool.tile([D, 2 * D], f32, name="nbrE")
            nc.scalar.copy(out=nbrE[:], in_=pse[:])

            NTt = dpool.tile([D, 2, D], f32, name="NTt")
            extT = dpool.tile([D, 2, D], f32, name="extT")
            for rep in range(2):
                pt2 = dppool.tile([D, D], f32, tag="ptD")
                nc.tensor.transpose(out=pt2[:], in_=nbrE[:, rep * D:(rep + 1) * D],
                                    identity=ident_sb[:])
                nc.vector.tensor_copy(out=NTt[:, rep, :], in_=pt2[:])
                nc.tensor.matmul(out=ps3s[rep][:], lhsT=VT_sb[:],
                                 rhs=NTt[:, rep, :],
                                 start=False, stop=True)
                nc.scalar.activation(out=extT[:, rep, :], in_=ps3s[rep][:],
                                     func=AF.Relu)

            # transpose back -> [g, d], softmax rows -> X (bf16)
            Xg = dpool.tile([D, 2, D], bf16, name="Xg")
            xe = dpool.tile([D, D], f32, name="xe")
            nmx2 = dpool.tile([D, 1], f32, name="nmx2")
            sm2 = dpool.tile([D, 1], f32, name="sm2")
            rs2 = dpool.tile([D, 1], f32, name="rs2")
            for rep in range(2):
                pt3 = dppool.tile([D, D], f32, tag="ptD")
                nc.tensor.transpose(out=pt3[:], in_=extT[:, rep, :],
                                    identity=ident_sb[:])
                gw = D if rep == 0 else GL - D
                nc.vector.tensor_reduce(out=nmx2[:gw, :], in_=pt3[:gw, :],
                                        axis=mybir.AxisListType.X,
                                        op=ALU.max, negate=True)
                nc.scalar.activation(out=xe[:gw, :], in_=pt3[:gw, :],
                                     func=AF.Exp, bias=nmx2[:gw, :],
                                     accum_out=sm2[:gw, :])
                nc.vector.reciprocal(out=rs2[:gw, :], in_=sm2[:gw, :])
                nc.vector.tensor_scalar_mul(out=Xg[:gw, rep, :],
                                            in0=xe[:gw, :],
                                            scalar1=rs2[:gw, :])
            nc.sync.dma_start(out=X_loc_dram[0:D, :], in_=Xg[:, 0, :])
            nc.sync.dma_start(out=X_loc_dram[D:GL, :], in_=Xg[:GL - D, 1, :])

        # ---- AllGather X shards ----
        nc.gpsimd.collective_compute(
            "AllGather", ALU.bypass,
            replica_groups=[list(range(NCORES))],
            ins=[X_loc_dram[:].opt()],
            outs=[X_full[:].opt()],
        )

        # =========================== Phase E ===========================
        with tc.tile_pool(name="epool", bufs=1) as epool, \
             tc.tile_pool(name="epsum", bufs=2, space="PSUM") as eppool:
            idx_pair_sb = epool.tile([D, 16], i16, name="idx_pair_sb")
            nc.sync.dma_start(out=idx_pair_sb[:], in_=idx_pair_in[:])
            gtp = epool.tile([D, 2, D], bf16, name="gtp")
            nc.gpsimd.dma_gather(
                out_ap=gtp[:], in_ap=X_full[:], idxs_ap=idx_pair_sb[:],
                num_idxs=256, num_idxs_reg=256, elem_size=D,
            )
            m = epool.tile([D, D], f32, name="m")
            nc.vector.tensor_mul(out=m[:], in0=gtp[:, 0, :], in1=gtp[:, 1, :])
            s = epool.tile([D, D], f32, name="s")
            nc.vector.tensor_add(out=s[:], in0=gtp[:, 0, :], in1=gtp[:, 1, :])

            mT = epool.tile([D, D], f32, name="mT")
            sT = epool.tile([D, D], f32, name="sT")
            for src, dst in ((m, mT), (s, sT)):
                ptE = eppool.tile([D, D], f32, tag="ptE")
                nc.tensor.transpose(out=ptE[:], in_=src[:], identity=ident_sb[:])
                nc.vector.tensor_copy(out=dst[:], in_=ptE[:])

            W1mT_sb = epool.tile([D, D], f32, name="W1mT_sb")
            nc.sync.dma_start(out=W1mT_sb[:], in_=W1mT[:])
            W1sT_sb = epool.tile([D, D], f32, name="W1sT_sb")
            nc.sync.dma_start(out=W1sT_sb[:], in_=W1sT[:])
            W2T_sb = epool.tile([D, 2], f32, name="W2T_sb")
            nc.sync.dma_start(out=W2T_sb[:], in_=W2T[:])
            b1_sb = epool.tile([D, 1], f32, name="b1_sb")
            nc.sync.dma_start(out=b1_sb[:], in_=b1_in[:])
            b2_sb = epool.tile([2, 1], f32, name="b2_sb")
            nc.sync.dma_start(out=b2_sb[:], in_=b2_in[:])

            ps4 = eppool.tile([D, D], f32, name="ps4")
            nc.tensor.matmul(out=ps4[:], lhsT=W1mT_sb[:], rhs=mT[:],
                             start=True, stop=False)
            nc.tensor.matmul(out=ps4[:], lhsT=W1sT_sb[:], rhs=sT[:],
                             start=False, stop=True)
            hT = epool.tile([D, D], f32, name="hT")
            nc.scalar.activation(out=hT[:], in_=ps4[:], func=AF.Relu,
                                 bias=b1_sb[:])

            ps5 = eppool.tile([2, D], f32, name="ps5")
            nc.tensor.matmul(out=ps5[:], lhsT=W2T_sb[:], rhs=hT[:],
                             start=True, stop=True)
            lgT = epool.tile([2, D], f32, name="lgT")
            nc.vector.tensor_scalar_add(out=lgT[:], in0=ps5[:], scalar1=b2_sb[:])

            ps6 = eppool.tile([D, 2], f32, name="ps6")
            nc.tensor.transpose(out=ps6[:], in_=lgT[:], identity=ident_sb[:2, :2])
            lg = epool.tile([D, 2], f32, name="lg")
            nc.vector.tensor_copy(out=lg[:], in_=ps6[:])

            nmx3 = epool.tile([D, 1], f32, name="nmx3")
            nc.vector.tensor_reduce(out=nmx3[:], in_=lg[:],
                                    axis=mybir.AxisListType.X,
                                    op=ALU.max, negate=True)
            ex3 = epool.tile([D, 2], f32, name="ex3")
            sm3 = epool.tile([D, 1], f32, name="sm3")
            nc.scalar.activation(out=ex3[:], in_=lg[:], func=AF.Exp,
                                 bias=nmx3[:], accum_out=sm3[:])
            rs3 = epool.tile([D, 1], f32, name="rs3")
            nc.vector.reciprocal(out=rs3[:], in_=sm3[:])
            nc.vector.tensor_scalar_mul(out=ex3[:], in0=ex3[:], scalar1=rs3[:])
            nc.sync.dma_start(out=out_dram[:], in_=ex3[:])

        cpool_cm.__exit__(None, None, None)
        for f in (_f0, _f1, _f2, _f3, _f4):
            f()

    nc.compile()
    return nc


def _prep_in_maps(inputs):
    batch = np.asarray(inputs["batch"])
    node_type = np.asarray(inputs["node_type"])
    nbr_type = np.asarray(inputs["nbr_type"])
    ext_nbr = np.asarray(inputs["ext_nbr"])
    impact = np.asarray(inputs["impact"], dtype=np.float32)
    W = np.asarray(inputs["W"], dtype=np.float32)
    M = np.asarray(inputs["M"], dtype=np.float32)
    U = np.asarray(inputs["U"], dtype=np.float32)
    V = np.asarray(inputs["V"], dtype=np.float32)
    W1 = np.asarray(inputs["W1"], dtype=np.float32)
    b1 = np.asarray(inputs["b1"], dtype=np.float32)
    W2 = np.asarray(inputs["W2"], dtype=np.float32)
    b2 = np.asarray(inputs["b2"], dtype=np.float32)

    ks = np.zeros((D, 8 * 16), dtype=np.float32)
    for bi in range(8):
        ks[:K, bi * 16 + 2 * bi] = 1.0
        ks[K:, bi * 16 + 2 * bi + 1] = 1.0

    import ml_dtypes
    ident = np.eye(D, dtype=np.float32)
    identh = ident.astype(ml_dtypes.bfloat16)
    shared = dict(
        impact_T=np.ascontiguousarray(impact.T),
        rhs_MW=np.ascontiguousarray(np.concatenate([M.T, W.T], axis=1)),
        UT=np.ascontiguousarray(U.T),
        VT=np.ascontiguousarray(V.T),
        W1mT=np.ascontiguousarray(W1[:, :D].T),
        W1sT=np.ascontiguousarray(W1[:, D:].T),
        W2T=np.ascontiguousarray(W2.T),
        b1=np.ascontiguousarray(b1.reshape(D, 1)),
        b2=np.ascontiguousarray(b2.reshape(2, 1)),
        ident=ident,
        identh=np.ascontiguousarray(identh),
        ks=ks,
    )

    in_maps = []
    for c in range(NCORES):
        g0 = c * GL
        nbr = nbr_type[g0:g0 + GL].reshape(NKL, DIN).astype(np.int64)
        slf = node_type[g0:g0 + GL].reshape(NKL).astype(np.int64)
        parts = []
        for lo, hi in _chunks():
            blocks = [nbr[lo:hi, j] for j in range(DIN)]
            blocks.append(NT + slf[lo:hi])
            parts.append(np.concatenate(blocks))
        idx_big = _wrap16(np.concatenate(parts))

        ex = np.zeros((DEXT, 256), np.int64)
        ex[:, :GL] = ext_nbr[g0:g0 + GL].T
        idx_ext = _wrap16(ex.reshape(-1))

        pair = np.concatenate([
            batch[c * BL:(c + 1) * BL, 0],
            batch[c * BL:(c + 1) * BL, 1],
        ])
        idx_pair = _wrap16(pair)

        m = dict(shared)
        m["idx_big"] = idx_big
        m["idx_ext"] = idx_ext
        m["idx_pair"] = idx_pair
        in_maps.append(m)
    return in_maps


def kernel(**inputs):
    in_maps = _prep_in_maps(inputs)
    if "nc" not in _PROGRAM_CACHE:
        _PROGRAM_CACHE["nc"] = build_program()
    nc = _PROGRAM_CACHE["nc"]

    from concourse import bass_utils
    res = bass_utils.run_bass_kernel_spmd(nc, in_maps, core_ids=list(range(NCORES)))
    out = np.concatenate([r["out"] for r in res.results], axis=0)
    return out.astype(np.float32)
